# revision 6
# baseline (speedup 1.0000x reference)
"""GatedDeltaNet mixer on 8 Trainium2 NeuronCores.

Sharding: data-parallel over batch (cores 0-3 = batch 0, cores 4-7 = batch 1),
tensor-parallel over heads within each group (4 heads/core). Per core:
q/k/v/g/beta projections (bf16), causal depthwise conv (DVE shifted fused ops),
per-head LN (ones-matmul broadcast stats), chunked delta-rule scan (chunk=128,
decay matrices from softplus/cumsum matmuls + ACT exp), silu gating, AllGather
of the gated output within each 4-core group, then a column-sharded output
projection. Host only transposes/shards/casts inputs and concatenates outputs.
"""

import sys

sys.path.insert(0, "/opt/trn_rl_repo")

import ml_dtypes
import numpy as np

import concourse.bass as bass
import concourse.mybir as mybir
import concourse.tile as tile
from concourse.bass_utils import run_bass_kernel_spmd
from concourse.masks import make_identity, make_lower_triangular, make_upper_triangular
from concourse.tile_sem_assignment import N_PROCS
from concourse.vector_clock import ScopedClock, VectorClock

def _split_sync_waits_json(bir_json: bytes) -> bytes:
    """Legalize BIR sync waits for this container's walrus build.

    The walrus here encodes at most one sync-wait command on a regular
    instruction (two on EventSemaphore). Tile's sem-assignment attaches the
    full wait set to the consuming instruction, so spill the excess onto
    EventSemaphore carriers inserted just before it on the same engine —
    the engine executes serially, so the conjunction is preserved.
    """
    import orjson

    d = orjson.loads(bir_json)
    n = 0
    for func in d.get("functions", []):
        for bb in func.get("blocks", []):
            out = []
            for inst in bb.get("instructions", []):
                si = inst.get("sync_info")
                if si:
                    ws = si.get("on_wait") or []
                    cap = 2 if inst.get("opcode") == "EventSemaphore" else 1
                    if len(ws) > cap:
                        for w in ws[:-cap]:
                            n += 1
                            out.append({
                                "debug": inst.get("debug"),
                                "engine": inst["engine"],
                                "ins": [],
                                "name": f"SWS-{n}",
                                "opcode": "EventSemaphore",
                                "outs": [],
                                "sync_info": {"on_update": [], "on_wait": [w]},
                            })
                        si["on_wait"] = ws[-cap:]
                out.append(inst)
            bb["instructions"] = out
    return orjson.dumps(d)


def _install_wait_split_hook():
    import concourse.bass2jax as _b2j
    import concourse.bass_utils as _bu

    if getattr(_bu, "_wait_split_installed", False):
        return
    _orig = _bu.compile_bir_kernel

    def _patched(bir_json, tmpdir, neff_name="file.neff"):
        return _orig(_split_sync_waits_json(bir_json), tmpdir, neff_name)

    _bu.compile_bir_kernel = _patched
    _b2j.compile_bir_kernel = _patched
    _bu._wait_split_installed = True


_install_wait_split_hook()

BF16 = mybir.dt.bfloat16
F32 = mybir.dt.float32
AF = mybir.ActivationFunctionType
OP = mybir.AluOpType
BF = ml_dtypes.bfloat16

B, L, D = 2, 2048, 2048
H, DK, DV, K = 16, 128, 128, 4
CH = 128               # scan chunk length
NTC = L // CH          # 16 chunks
NK = D // 128          # 16 contraction tiles
HL = 4                 # heads per core
FSH = HL * DK          # 512 local feature columns
NCORES = 8
GROUPS = [[0, 1, 2, 3], [4, 5, 6, 7]]
EPS = 1e-5
NEG = -1.0e9


class _SplitDrainTC(tile.TileContext):
    """TileContext whose exit drain splits its semaphore waits.

    The walrus build here caps sync-wait commands at 1 per regular
    instruction; Tile's stock exit drain carries one wait per logical proc
    and fails to compile. Waits are moved onto a chain of NOPs instead.
    """

    def _drain_and_barrier(self, tick_clock, wait_clock):
        g = tick_clock.global_clock
        vals = [g[p] for p in range(N_PROCS)]
        for p in range(N_PROCS):
            if vals[p] <= 0:
                continue
            cvals = [vals[q] if q == p else 0 for q in range(N_PROCS)]
            d = self.nc.sync.nop(nofuse=True)
            wait_clock.add_sem_waits(d.ins, ScopedClock({None: VectorClock(cvals)}))
        self.nc.sync.drain()

        self.nc.all_engine_barrier()
        assert self.sems is not None
        popped = self.nc._tile_sem_poison_stack.pop()
        assert popped is self._sem_poison
        self.nc.clear_and_free_semaphores(list(self.sems.allocated().values()))
        self.nc.all_engine_barrier()


def build_kernel() -> bass.Bass:
    nc = bass.Bass()

    hT = nc.declare_dram_parameter("hT", [D, L], BF16, isOutput=False)
    wq = nc.declare_dram_parameter("wq", [D, FSH], BF16, isOutput=False)
    wk = nc.declare_dram_parameter("wk", [D, FSH], BF16, isOutput=False)
    wg = nc.declare_dram_parameter("wg", [D, FSH], BF16, isOutput=False)
    wv = nc.declare_dram_parameter("wv", [D, FSH], BF16, isOutput=False)
    wb = nc.declare_dram_parameter("wb", [D, HL], BF16, isOutput=False)
    wo = nc.declare_dram_parameter("wo", [H * DV, FSH], BF16, isOutput=False)
    qcw = nc.declare_dram_parameter("qcw", [128, HL * K], F32, isOutput=False)
    kcw = nc.declare_dram_parameter("kcw", [128, HL * K], F32, isOutput=False)
    qcb = nc.declare_dram_parameter("qcb", [128, HL], F32, isOutput=False)
    kcb = nc.declare_dram_parameter("kcb", [128, HL], F32, isOutput=False)
    qnw = nc.declare_dram_parameter("qnw", [128, 1], F32, isOutput=False)
    qnb = nc.declare_dram_parameter("qnb", [128, 1], F32, isOutput=False)
    knw = nc.declare_dram_parameter("knw", [128, 1], F32, isOutput=False)
    knb = nc.declare_dram_parameter("knb", [128, 1], F32, isOutput=False)
    bbb = nc.declare_dram_parameter("bbb", [128, HL], F32, isOutput=False)
    out = nc.declare_dram_parameter("out", [L, FSH], F32, isOutput=True)

    og_d = nc.dram_tensor("og_d", [FSH, L], BF16)
    og_all = nc.dram_tensor("og_all", [H * DV, L], BF16)

    with _SplitDrainTC(nc) as tc:
        with tc.tile_pool(name="ps", bufs=8, space="PSUM") as ps:
            _build_main(nc, tc, ps, locals())
    return nc


def _build_main(nc, tc, ps, t):
    hT, wq, wk, wg, wv, wb = t["hT"], t["wq"], t["wk"], t["wg"], t["wv"], t["wb"]
    wo, qcw, kcw, qcb, kcb = t["wo"], t["qcw"], t["kcw"], t["qcb"], t["kcb"]
    qnw, qnb, knw, knb, bbb = t["qnw"], t["qnb"], t["knw"], t["knb"], t["bbb"]
    out, og_d, og_all = t["out"], t["og_d"], t["og_all"]

    with tc.tile_pool(name="wp", bufs=1) as wp:
        # ---- persistent tiles -------------------------------------------
        def load16(param, name, width):
            tiles = []
            for i in range(NK):
                tt = wp.tile([128, width], BF16, tag=f"{name}{i}")
                nc.sync.dma_start(out=tt, in_=param[i * 128 : (i + 1) * 128, :])
                tiles.append(tt)
            return tiles

        hT_t = load16(hT, "hT", L)
        wq_t = load16(wq, "wq", FSH)
        wk_t = load16(wk, "wk", FSH)
        wg_t = load16(wg, "wg", FSH)

        ones_sc = wp.tile([128, 128], BF16, tag="ones_sc")
        nc.gpsimd.memset(ones_sc, 1.0 / 128.0)
        ones_one = wp.tile([128, 128], BF16, tag="ones_one")
        nc.gpsimd.memset(ones_one, 1.0)
        negu = wp.tile([128, 128], BF16, tag="negu")
        make_upper_triangular(nc, negu, val=-1.0, diag=True)
        ident = wp.tile([128, 128], BF16, tag="ident")
        make_identity(nc, ident)
        maskc = wp.tile([128, 128], BF16, tag="maskc")
        make_lower_triangular(nc, maskc, val=NEG, diag=False)

        qcw_t = wp.tile([128, HL * K], F32, tag="qcw")
        nc.sync.dma_start(out=qcw_t, in_=qcw[:, :])
        kcw_t = wp.tile([128, HL * K], F32, tag="kcw")
        nc.sync.dma_start(out=kcw_t, in_=kcw[:, :])
        qcb_t = wp.tile([128, HL], F32, tag="qcb")
        nc.sync.dma_start(out=qcb_t, in_=qcb[:, :])
        kcb_t = wp.tile([128, HL], F32, tag="kcb")
        nc.sync.dma_start(out=kcb_t, in_=kcb[:, :])
        qnw_t = wp.tile([128, 1], F32, tag="qnw")
        nc.sync.dma_start(out=qnw_t, in_=qnw[:, :])
        qnb_t = wp.tile([128, 1], F32, tag="qnb")
        nc.sync.dma_start(out=qnb_t, in_=qnb[:, :])
        knw_t = wp.tile([128, 1], F32, tag="knw")
        nc.sync.dma_start(out=knw_t, in_=knw[:, :])
        knb_t = wp.tile([128, 1], F32, tag="knb")
        nc.sync.dma_start(out=knb_t, in_=knb[:, :])

        eps_t = wp.tile([128, 1], F32, tag="eps")
        nc.gpsimd.memset(eps_t, EPS)
        v_all = wp.tile([128, NTC * FSH], BF16, tag="v_all")
        sp_all = wp.tile([128, NTC * HL], F32, tag="sp_all")
        spn_all = wp.tile([128, NTC * HL], F32, tag="spn_all")

        # ---- phase 1: v (time-major) + beta softplus --------------------
        with tc.tile_pool(name="vp", bufs=1) as vp, \
             tc.tile_pool(name="vp2", bufs=2) as vp2:
            wv_t = []
            for i in range(NK):
                tt = vp.tile([128, FSH], BF16, tag=f"wv{i}")
                nc.sync.dma_start(out=tt, in_=wv[i * 128 : (i + 1) * 128, :])
                wv_t.append(tt)
            wb_t = []
            for i in range(NK):
                tt = vp.tile([128, HL], BF16, tag=f"wb{i}")
                nc.sync.dma_start(out=tt, in_=wb[i * 128 : (i + 1) * 128, :])
                wb_t.append(tt)
            bbb_t = vp.tile([128, HL], F32, tag="bbb")
            nc.sync.dma_start(out=bbb_t, in_=bbb[:, :])

            for ci in range(NTC):
                csl = slice(ci * 128, (ci + 1) * 128)
                vps = ps.tile([128, FSH], F32, tag="ps")
                for kk in range(NK):
                    nc.tensor.matmul(vps, hT_t[kk][:, csl], wv_t[kk],
                                     start=(kk == 0), stop=(kk == NK - 1))
                nc.scalar.activation(v_all[:, ci * FSH : (ci + 1) * FSH], vps, AF.Copy)
                bps = ps.tile([128, HL], F32, tag="ps")
                for kk in range(NK):
                    nc.tensor.matmul(bps, hT_t[kk][:, csl], wb_t[kk],
                                     start=(kk == 0), stop=(kk == NK - 1))
                zb = vp2.tile([128, HL], F32, tag="zb")
                nc.vector.tensor_tensor(zb, bps, bbb_t, OP.add)
                bsl = slice(ci * HL, (ci + 1) * HL)
                # softplus(z) = ln(1 + exp(z)); z stays well under overflow range
                e1 = vp2.tile([128, HL], F32, tag="e1")
                nc.scalar.activation(e1, zb, AF.Exp)
                nc.scalar.activation(sp_all[:, bsl], e1, AF.Ln, bias=1.0)
                e2 = vp2.tile([128, HL], F32, tag="e2")
                nc.scalar.activation(e2, zb, AF.Exp, scale=-1.0)
                nc.scalar.activation(spn_all[:, bsl], e2, AF.Ln, bias=1.0)

        # ---- phase 2: per-head prep + scan ------------------------------
        with tc.tile_pool(name="wk1", bufs=1) as wk1, \
             tc.tile_pool(name="wk2", bufs=2) as wk2, \
             tc.tile_pool(name="wk3", bufs=3) as wk3:
            for h in range(HL):
                hsl = slice(h * 128, (h + 1) * 128)
                qln = wk1.tile([128, L], BF16, tag="qln")
                kln = wk1.tile([128, L], BF16, tag="kln")
                for dst, w_t, cw, cb, lw, lb in (
                    (qln, wq_t, qcw_t, qcb_t, qnw_t, qnb_t),
                    (kln, wk_t, kcw_t, kcb_t, knw_t, knb_t),
                ):
                    xpad = wk1.tile([128, 4 + L], BF16, tag="xpad")
                    nc.gpsimd.memset(xpad[:, 0:4], 0.0)
                    for tt2 in range(4):
                        pps = ps.tile([128, 512], F32, tag="ps")
                        tsl = slice(tt2 * 512, (tt2 + 1) * 512)
                        for kk in range(NK):
                            nc.tensor.matmul(pps, w_t[kk][:, hsl], hT_t[kk][:, tsl],
                                             start=(kk == 0), stop=(kk == NK - 1))
                        nc.scalar.activation(xpad[:, 4 + tt2 * 512 : 4 + (tt2 + 1) * 512],
                                             pps, AF.Copy)
                    y = wk1.tile([128, L], BF16, tag="convy")
                    nc.vector.tensor_scalar_mul(y, xpad[:, 1 : 1 + L],
                                                cw[:, h * K : h * K + 1])
                    for s in (1, 2, 3):
                        nc.vector.scalar_tensor_tensor(
                            y, xpad[:, 1 + s : 1 + s + L],
                            cw[:, h * K + s : h * K + s + 1], y, OP.mult, OP.add)
                    sil = wk1.tile([128, L], BF16, tag="sil")
                    nc.scalar.activation(sil, y, AF.Silu, bias=cb[:, h : h + 1])
                    for tt2 in range(4):
                        tsl = slice(tt2 * 512, (tt2 + 1) * 512)
                        sq = wk2.tile([128, 512], BF16, tag="sq")
                        nc.scalar.square(sq, sil[:, tsl])
                        mups = ps.tile([128, 512], F32, tag="ps")
                        nc.tensor.matmul(mups, ones_sc, sil[:, tsl], start=True, stop=True)
                        sqps = ps.tile([128, 512], F32, tag="ps")
                        nc.tensor.matmul(sqps, ones_sc, sq, start=True, stop=True)
                        m2 = wk2.tile([128, 512], F32, tag="m2")
                        nc.scalar.square(m2, mups)
                        vt = wk2.tile([128, 512], F32, tag="vt")
                        nc.vector.tensor_tensor(vt, sqps, m2, OP.subtract)
                        nc.scalar.activation(vt, vt, AF.Ln, bias=eps_t)
                        r0 = wk2.tile([128, 512], BF16, tag="r0")
                        nc.scalar.activation(r0, vt, AF.Exp, scale=-0.5)
                        r1 = wk2.tile([128, 512], BF16, tag="r1")
                        nc.vector.tensor_scalar_mul(r1, r0, lw)
                        s1 = wk2.tile([128, 512], BF16, tag="s1")
                        nc.vector.scalar_tensor_tensor(s1, mups, -1.0, r1,
                                                       OP.mult, OP.mult)
                        t1 = wk2.tile([128, 512], BF16, tag="t1")
                        nc.vector.tensor_tensor(t1, sil[:, tsl], r1, OP.mult)
                        nc.vector.scalar_tensor_tensor(dst[:, tsl], t1, lb, s1,
                                                       OP.add, OP.add)

                kln_tm = wk1.tile([128, L], BF16, tag="klntm")
                for ci in range(NTC):
                    csl = slice(ci * 128, (ci + 1) * 128)
                    nc.scalar.dma_start_transpose(kln_tm[:, csl], kln[:, csl])

                gsil = wk1.tile([128, L], BF16, tag="gsil")
                for tt2 in range(4):
                    tsl = slice(tt2 * 512, (tt2 + 1) * 512)
                    pps = ps.tile([128, 512], F32, tag="ps")
                    for kk in range(NK):
                        nc.tensor.matmul(pps, wg_t[kk][:, hsl], hT_t[kk][:, tsl],
                                         start=(kk == 0), stop=(kk == NK - 1))
                    nc.scalar.activation(gsil[:, tsl], pps, AF.Silu)

                # ---- chunked scan ----
                S_bf = wk1.tile([128, 128], BF16, tag="sbf")
                nc.gpsimd.memset(S_bf, 0.0)
                og_h = wk1.tile([128, L], BF16, tag="ogh")
                for ci in range(NTC):
                    csl = slice(ci * 128, (ci + 1) * 128)
                    vsl = slice(ci * FSH + h * 128, ci * FSH + (h + 1) * 128)
                    spc = sp_all[:, ci * HL + h : ci * HL + h + 1]
                    spnc = spn_all[:, ci * HL + h : ci * HL + h + 1]

                    Yt = wk3.tile([128, 128], BF16, tag="Y")
                    nc.vector.tensor_scalar_mul(Yt, negu, spc)
                    gps = ps.tile([128, 128], F32, tag="ps")
                    nc.tensor.matmul(gps, ones_one, Yt, start=True, stop=False,
                                     skip_group_check=True)
                    gtps = ps.tile([128, 128], F32, tag="ps")
                    nc.tensor.matmul(gtps, Yt, ones_one, start=True, stop=True,
                                     skip_group_check=True)
                    ptile = wk3.tile([128, 128], F32, tag="pt")
                    nc.scalar.activation(ptile, gps, AF.Exp)
                    scol = wk3.tile([128, 1], F32, tag="scol")
                    nc.vector.scalar_tensor_tensor(scol, spnc, -1.0, gtps[:, 0:1],
                                                   OP.mult, OP.subtract)
                    nc.tensor.matmul(gps, ident, maskc, start=False, stop=True,
                                     skip_group_check=True)
                    dexp = wk3.tile([128, 128], F32, tag="dexp")
                    nc.scalar.activation(dexp, gps, AF.Exp, bias=scol)

                    atps = ps.tile([128, 128], F32, tag="ps")
                    nc.tensor.matmul(atps, kln[:, csl], qln[:, csl],
                                     start=True, stop=True)
                    ats = wk3.tile([128, 128], BF16, tag="ats")
                    nc.vector.tensor_tensor(ats, atps, dexp, OP.mult)
                    qs = wk3.tile([128, 128], BF16, tag="qs")
                    nc.vector.tensor_tensor(qs, qln[:, csl], ptile, OP.mult)
                    kw = wk3.tile([128, 128], BF16, tag="kw")
                    nc.vector.tensor_scalar_mul(kw, kln_tm[:, csl], dexp[:, 127:128])

                    ops_ = ps.tile([128, 128], F32, tag="ps")
                    nc.tensor.matmul(ops_, v_all[:, vsl], ats, start=True, stop=False,
                                     skip_group_check=True)
                    nc.tensor.matmul(ops_, S_bf, qs, start=False, stop=True,
                                     skip_group_check=True)
                    nc.vector.tensor_tensor(og_h[:, csl], ops_, gsil[:, csl], OP.mult)

                    sd = wk3.tile([128, 128], BF16, tag="sd")
                    nc.vector.tensor_scalar_mul(sd, ident, ptile[:, 127:128])
                    sps = ps.tile([128, 128], F32, tag="ps")
                    nc.tensor.matmul(sps, sd, S_bf, start=True, stop=False,
                                     skip_group_check=True)
                    nc.tensor.matmul(sps, kw, v_all[:, vsl], start=False, stop=True,
                                     skip_group_check=True)
                    nc.vector.tensor_copy(S_bf, sps)

                nc.sync.dma_start(out=og_d[h * 128 : (h + 1) * 128, :], in_=og_h)

        # ---- phase 3: AllGather ----------------------------------------
        nc.gpsimd.collective_compute(
            "AllGather", OP.bypass, replica_groups=GROUPS,
            ins=[og_d[:, :]], outs=[og_all[:, :]],
        )

    # ---- phase 4: out projection (wp freed) ----------------------------
    with tc.tile_pool(name="p4", bufs=1) as p4, \
         tc.tile_pool(name="p4b", bufs=3) as p4b:
        wo_t = []
        og_r = []
        for i in range(H * DV // 128):
            tt = p4.tile([128, FSH], BF16, tag=f"wo{i}")
            nc.sync.dma_start(out=tt, in_=wo[i * 128 : (i + 1) * 128, :])
            wo_t.append(tt)
            rr = p4.tile([128, L], BF16, tag=f"ogr{i}")
            nc.sync.dma_start(out=rr, in_=og_all[i * 128 : (i + 1) * 128, :])
            og_r.append(rr)
        for ci in range(NTC):
            csl = slice(ci * 128, (ci + 1) * 128)
            ops_ = ps.tile([128, FSH], F32, tag="ps")
            for ff in range(H * DV // 128):
                nc.tensor.matmul(ops_, og_r[ff][:, csl], wo_t[ff],
                                 start=(ff == 0), stop=(ff == H * DV // 128 - 1))
            outt = p4b.tile([128, FSH], F32, tag="outt")
            nc.scalar.activation(outt, ops_, AF.Copy)
            nc.sync.dma_start(out=out[csl, :], in_=outt)


def prep_inputs(inputs):
    """Shard/transpose/cast full inputs into 8 per-core in_maps."""
    f = {k: np.asarray(v, dtype=np.float32) for k, v in inputs.items()}
    h = f["hidden_states"]

    hT_b = [np.ascontiguousarray(h[b].T).astype(BF) for b in range(B)]
    in_maps = []
    for c in range(NCORES):
        b, r = c // 4, c % 4
        cols = slice(r * FSH, (r + 1) * FSH)

        def convw(w):
            m = w[cols].reshape(HL, 128, K).transpose(1, 0, 2).reshape(128, HL * K)
            return np.ascontiguousarray(m).astype(np.float32)

        def convb(bias):
            return np.ascontiguousarray(bias[cols].reshape(HL, 128).T).astype(np.float32)

        m = {
            "hT": hT_b[b],
            "wq": np.ascontiguousarray(f["Wq"][:, cols]).astype(BF),
            "wk": np.ascontiguousarray(f["Wk"][:, cols]).astype(BF),
            "wg": np.ascontiguousarray(f["Wg"][:, cols]).astype(BF),
            "wv": np.ascontiguousarray(f["Wv"][:, cols]).astype(BF),
            "wb": np.ascontiguousarray(f["Wb"][:, 4 * r : 4 * r + 4]).astype(BF),
            "wo": np.ascontiguousarray(f["Wo"][:, cols]).astype(BF),
            "qcw": convw(f["qconv_w"]),
            "kcw": convw(f["kconv_w"]),
            "qcb": convb(f["qconv_b"]),
            "kcb": convb(f["kconv_b"]),
            "qnw": f["qn_w"].reshape(128, 1).astype(np.float32),
            "qnb": f["qn_b"].reshape(128, 1).astype(np.float32),
            "knw": f["kn_w"].reshape(128, 1).astype(np.float32),
            "knb": f["kn_b"].reshape(128, 1).astype(np.float32),
            "bbb": np.ascontiguousarray(
                np.broadcast_to(f["bb"][4 * r : 4 * r + 4], (128, HL))
            ).astype(np.float32),
        }
        in_maps.append(m)
    return in_maps


_NC_CACHE = {}


def get_nc():
    if "nc" not in _NC_CACHE:
        _NC_CACHE["nc"] = build_kernel()
    return _NC_CACHE["nc"]


def assemble(results):
    full = np.empty((B, L, D), np.float32)
    for c in range(NCORES):
        b, r = c // 4, c % 4
        full[b][:, r * FSH : (r + 1) * FSH] = results[c]["out"]
    return full


def kernel(**inputs) -> np.ndarray:
    nc = get_nc()
    in_maps = prep_inputs(inputs)
    res = run_bass_kernel_spmd(nc, in_maps, list(range(NCORES)))
    return assemble(res.results)


# revision 7
# speedup vs baseline: 56.1674x; 56.1674x over previous
"""GatedDeltaNet mixer on 8 Trainium2 NeuronCores.

Sharding: data-parallel over batch (cores 0-3 = batch 0, cores 4-7 = batch 1),
tensor-parallel over heads within each group (4 heads/core). Per core:
q/k/v/g/beta projections (bf16), causal depthwise conv (DVE shifted fused ops),
per-head LN (ones-matmul broadcast stats), chunked delta-rule scan (chunk=128,
decay matrices from softplus/cumsum matmuls + ACT exp), silu gating, AllGather
of the gated output within each 4-core group, then a column-sharded output
projection. Host only transposes/shards/casts inputs and concatenates outputs.
"""

import sys

sys.path.insert(0, "/opt/trn_rl_repo")

import ml_dtypes
import numpy as np

import concourse.bass as bass
import concourse.mybir as mybir
import concourse.tile as tile
from concourse.bass_utils import run_bass_kernel_spmd
from concourse.masks import make_identity, make_lower_triangular, make_upper_triangular
from concourse.tile_sem_assignment import N_PROCS
from concourse.vector_clock import ScopedClock, VectorClock

def _split_sync_waits_json(bir_json: bytes) -> bytes:
    """Legalize BIR sync waits for this container's walrus build.

    The walrus here encodes at most one sync-wait command on a regular
    instruction (two on EventSemaphore). Tile's sem-assignment attaches the
    full wait set to the consuming instruction, so spill the excess onto
    EventSemaphore carriers inserted just before it on the same engine —
    the engine executes serially, so the conjunction is preserved.
    """
    import orjson

    d = orjson.loads(bir_json)
    n = 0
    for func in d.get("functions", []):
        for bb in func.get("blocks", []):
            out = []
            for inst in bb.get("instructions", []):
                si = inst.get("sync_info")
                if si:
                    ws = si.get("on_wait") or []
                    cap = 2 if inst.get("opcode") == "EventSemaphore" else 1
                    if len(ws) > cap:
                        for w in ws[:-cap]:
                            n += 1
                            out.append({
                                "debug": inst.get("debug"),
                                "engine": inst["engine"],
                                "ins": [],
                                "name": f"SWS-{n}",
                                "opcode": "EventSemaphore",
                                "outs": [],
                                "sync_info": {"on_update": [], "on_wait": [w]},
                            })
                        si["on_wait"] = ws[-cap:]
                out.append(inst)
            bb["instructions"] = out
    return orjson.dumps(d)


def _install_wait_split_hook():
    import concourse.bass2jax as _b2j
    import concourse.bass_utils as _bu

    if getattr(_bu, "_wait_split_installed", False):
        return
    _orig = _bu.compile_bir_kernel

    def _patched(bir_json, tmpdir, neff_name="file.neff"):
        return _orig(_split_sync_waits_json(bir_json), tmpdir, neff_name)

    _bu.compile_bir_kernel = _patched
    _b2j.compile_bir_kernel = _patched
    _bu._wait_split_installed = True


_install_wait_split_hook()

BF16 = mybir.dt.bfloat16
F32 = mybir.dt.float32
AF = mybir.ActivationFunctionType
OP = mybir.AluOpType
BF = ml_dtypes.bfloat16

B, L, D = 2, 2048, 2048
H, DK, DV, K = 16, 128, 128, 4
CH = 128               # scan chunk length
NTC = L // CH          # 16 chunks
NK = D // 128          # 16 contraction tiles
HL = 4                 # heads per core
FSH = HL * DK          # 512 local feature columns
NCORES = 8
GROUPS = [[0, 1, 2, 3], [4, 5, 6, 7]]
EPS = 1e-5
NEG = -1.0e9


class _SplitDrainTC(tile.TileContext):
    """TileContext whose exit drain splits its semaphore waits.

    The walrus build here caps sync-wait commands at 1 per regular
    instruction; Tile's stock exit drain carries one wait per logical proc
    and fails to compile. Waits are moved onto a chain of NOPs instead.
    """

    def _drain_and_barrier(self, tick_clock, wait_clock):
        g = tick_clock.global_clock
        vals = [g[p] for p in range(N_PROCS)]
        for p in range(N_PROCS):
            if vals[p] <= 0:
                continue
            cvals = [vals[q] if q == p else 0 for q in range(N_PROCS)]
            d = self.nc.sync.nop(nofuse=True)
            wait_clock.add_sem_waits(d.ins, ScopedClock({None: VectorClock(cvals)}))
        self.nc.sync.drain()

        self.nc.all_engine_barrier()
        assert self.sems is not None
        popped = self.nc._tile_sem_poison_stack.pop()
        assert popped is self._sem_poison
        self.nc.clear_and_free_semaphores(list(self.sems.allocated().values()))
        self.nc.all_engine_barrier()


def build_kernel(reps: int = 1) -> bass.Bass:
    nc = bass.Bass()

    hT = nc.declare_dram_parameter("hT", [D, L], BF16, isOutput=False)
    wq = nc.declare_dram_parameter("wq", [D, FSH], BF16, isOutput=False)
    wk = nc.declare_dram_parameter("wk", [D, FSH], BF16, isOutput=False)
    wg = nc.declare_dram_parameter("wg", [D, FSH], BF16, isOutput=False)
    wv = nc.declare_dram_parameter("wv", [D, FSH], BF16, isOutput=False)
    wb = nc.declare_dram_parameter("wb", [D, HL], BF16, isOutput=False)
    wo = nc.declare_dram_parameter("wo", [H * DV, FSH], BF16, isOutput=False)
    qcw = nc.declare_dram_parameter("qcw", [128, HL * K], F32, isOutput=False)
    kcw = nc.declare_dram_parameter("kcw", [128, HL * K], F32, isOutput=False)
    qcb = nc.declare_dram_parameter("qcb", [128, HL], F32, isOutput=False)
    kcb = nc.declare_dram_parameter("kcb", [128, HL], F32, isOutput=False)
    qnw = nc.declare_dram_parameter("qnw", [128, 1], F32, isOutput=False)
    qnb = nc.declare_dram_parameter("qnb", [128, 1], F32, isOutput=False)
    knw = nc.declare_dram_parameter("knw", [128, 1], F32, isOutput=False)
    knb = nc.declare_dram_parameter("knb", [128, 1], F32, isOutput=False)
    bbb = nc.declare_dram_parameter("bbb", [128, HL], F32, isOutput=False)
    out = nc.declare_dram_parameter("out", [L, FSH], F32, isOutput=True)

    og_d = nc.dram_tensor("og_d", [FSH, L], BF16)
    og_all = nc.dram_tensor("og_all", [H * DV, L], BF16)

    with _SplitDrainTC(nc) as tc:
        with tc.tile_pool(name="ps", bufs=8, space="PSUM") as ps:
            for _rep in range(reps):
                _build_main(nc, tc, ps, locals())
    return nc


def _build_main(nc, tc, ps, t):
    hT, wq, wk, wg, wv, wb = t["hT"], t["wq"], t["wk"], t["wg"], t["wv"], t["wb"]
    wo, qcw, kcw, qcb, kcb = t["wo"], t["qcw"], t["kcw"], t["qcb"], t["kcb"]
    qnw, qnb, knw, knb, bbb = t["qnw"], t["qnb"], t["knw"], t["knb"], t["bbb"]
    out, og_d, og_all = t["out"], t["og_d"], t["og_all"]

    with tc.tile_pool(name="wp", bufs=1) as wp:
        # ---- persistent tiles -------------------------------------------
        def load16(param, name, width):
            tiles = []
            for i in range(NK):
                tt = wp.tile([128, width], BF16, tag=f"{name}{i}")
                nc.sync.dma_start(out=tt, in_=param[i * 128 : (i + 1) * 128, :])
                tiles.append(tt)
            return tiles

        hT_t = load16(hT, "hT", L)
        wq_t = load16(wq, "wq", FSH)
        wk_t = load16(wk, "wk", FSH)
        wg_t = load16(wg, "wg", FSH)

        ones_sc = wp.tile([128, 128], BF16, tag="ones_sc")
        nc.gpsimd.memset(ones_sc, 1.0 / 128.0)
        ones_one = wp.tile([128, 128], BF16, tag="ones_one")
        nc.gpsimd.memset(ones_one, 1.0)
        negu = wp.tile([128, 128], BF16, tag="negu")
        make_upper_triangular(nc, negu, val=-1.0, diag=True)
        ident = wp.tile([128, 128], BF16, tag="ident")
        make_identity(nc, ident)
        maskc = wp.tile([128, 128], BF16, tag="maskc")
        make_lower_triangular(nc, maskc, val=NEG, diag=False)

        qcw_t = wp.tile([128, HL * K], F32, tag="qcw")
        nc.sync.dma_start(out=qcw_t, in_=qcw[:, :])
        kcw_t = wp.tile([128, HL * K], F32, tag="kcw")
        nc.sync.dma_start(out=kcw_t, in_=kcw[:, :])
        qcb_t = wp.tile([128, HL], F32, tag="qcb")
        nc.sync.dma_start(out=qcb_t, in_=qcb[:, :])
        kcb_t = wp.tile([128, HL], F32, tag="kcb")
        nc.sync.dma_start(out=kcb_t, in_=kcb[:, :])
        qnw_t = wp.tile([128, 1], F32, tag="qnw")
        nc.sync.dma_start(out=qnw_t, in_=qnw[:, :])
        qnb_t = wp.tile([128, 1], F32, tag="qnb")
        nc.sync.dma_start(out=qnb_t, in_=qnb[:, :])
        knw_t = wp.tile([128, 1], F32, tag="knw")
        nc.sync.dma_start(out=knw_t, in_=knw[:, :])
        knb_t = wp.tile([128, 1], F32, tag="knb")
        nc.sync.dma_start(out=knb_t, in_=knb[:, :])

        eps_t = wp.tile([128, 1], F32, tag="eps")
        nc.gpsimd.memset(eps_t, EPS)
        v_all = wp.tile([128, NTC * FSH], BF16, tag="v_all")
        sp_all = wp.tile([128, NTC * HL], F32, tag="sp_all")
        spn_all = wp.tile([128, NTC * HL], F32, tag="spn_all")

        # ---- phase 1: v (time-major) + beta softplus --------------------
        with tc.tile_pool(name="vp", bufs=1) as vp, \
             tc.tile_pool(name="vp2", bufs=2) as vp2:
            wv_t = []
            for i in range(NK):
                tt = vp.tile([128, FSH], BF16, tag=f"wv{i}")
                nc.sync.dma_start(out=tt, in_=wv[i * 128 : (i + 1) * 128, :])
                wv_t.append(tt)
            wb_t = []
            for i in range(NK):
                tt = vp.tile([128, HL], BF16, tag=f"wb{i}")
                nc.sync.dma_start(out=tt, in_=wb[i * 128 : (i + 1) * 128, :])
                wb_t.append(tt)
            bbb_t = vp.tile([128, HL], F32, tag="bbb")
            nc.sync.dma_start(out=bbb_t, in_=bbb[:, :])

            for ci in range(NTC):
                csl = slice(ci * 128, (ci + 1) * 128)
                vps = ps.tile([128, FSH], F32, tag="ps")
                for kk in range(NK):
                    nc.tensor.matmul(vps, hT_t[kk][:, csl], wv_t[kk],
                                     start=(kk == 0), stop=(kk == NK - 1))
                nc.scalar.activation(v_all[:, ci * FSH : (ci + 1) * FSH], vps, AF.Copy)
                bps = ps.tile([128, HL], F32, tag="ps")
                for kk in range(NK):
                    nc.tensor.matmul(bps, hT_t[kk][:, csl], wb_t[kk],
                                     start=(kk == 0), stop=(kk == NK - 1))
                zb = vp2.tile([128, HL], F32, tag="zb")
                nc.vector.tensor_tensor(zb, bps, bbb_t, OP.add)
                bsl = slice(ci * HL, (ci + 1) * HL)
                # softplus(z) = ln(1 + exp(z)); z stays well under overflow range
                e1 = vp2.tile([128, HL], F32, tag="e1")
                nc.scalar.activation(e1, zb, AF.Exp)
                nc.scalar.activation(sp_all[:, bsl], e1, AF.Ln, bias=1.0)
                e2 = vp2.tile([128, HL], F32, tag="e2")
                nc.scalar.activation(e2, zb, AF.Exp, scale=-1.0)
                nc.scalar.activation(spn_all[:, bsl], e2, AF.Ln, bias=1.0)

        # ---- phase 2: per-head prep + scan ------------------------------
        with tc.tile_pool(name="wk1", bufs=1) as wk1, \
             tc.tile_pool(name="wk2", bufs=2) as wk2, \
             tc.tile_pool(name="wk3", bufs=3) as wk3:
            for h in range(HL):
                hsl = slice(h * 128, (h + 1) * 128)
                qln = wk1.tile([128, L], BF16, tag="qln")
                kln = wk1.tile([128, L], BF16, tag="kln")
                for dst, w_t, cw, cb, lw, lb in (
                    (qln, wq_t, qcw_t, qcb_t, qnw_t, qnb_t),
                    (kln, wk_t, kcw_t, kcb_t, knw_t, knb_t),
                ):
                    xpad = wk1.tile([128, 4 + L], BF16, tag="xpad")
                    nc.gpsimd.memset(xpad[:, 0:4], 0.0)
                    for tt2 in range(4):
                        pps = ps.tile([128, 512], F32, tag="ps")
                        tsl = slice(tt2 * 512, (tt2 + 1) * 512)
                        for kk in range(NK):
                            nc.tensor.matmul(pps, w_t[kk][:, hsl], hT_t[kk][:, tsl],
                                             start=(kk == 0), stop=(kk == NK - 1))
                        nc.scalar.activation(xpad[:, 4 + tt2 * 512 : 4 + (tt2 + 1) * 512],
                                             pps, AF.Copy)
                    y = wk1.tile([128, L], BF16, tag="convy")
                    nc.vector.tensor_scalar_mul(y, xpad[:, 1 : 1 + L],
                                                cw[:, h * K : h * K + 1])
                    for s in (1, 2, 3):
                        nc.vector.scalar_tensor_tensor(
                            y, xpad[:, 1 + s : 1 + s + L],
                            cw[:, h * K + s : h * K + s + 1], y, OP.mult, OP.add)
                    sil = wk1.tile([128, L], BF16, tag="sil")
                    nc.scalar.activation(sil, y, AF.Silu, bias=cb[:, h : h + 1])
                    for tt2 in range(4):
                        tsl = slice(tt2 * 512, (tt2 + 1) * 512)
                        sq = wk2.tile([128, 512], BF16, tag="sq")
                        nc.scalar.square(sq, sil[:, tsl])
                        mups = ps.tile([128, 512], F32, tag="ps")
                        nc.tensor.matmul(mups, ones_sc, sil[:, tsl], start=True, stop=True)
                        sqps = ps.tile([128, 512], F32, tag="ps")
                        nc.tensor.matmul(sqps, ones_sc, sq, start=True, stop=True)
                        m2 = wk2.tile([128, 512], F32, tag="m2")
                        nc.scalar.square(m2, mups)
                        vt = wk2.tile([128, 512], F32, tag="vt")
                        nc.vector.tensor_tensor(vt, sqps, m2, OP.subtract)
                        nc.scalar.activation(vt, vt, AF.Ln, bias=eps_t)
                        r0 = wk2.tile([128, 512], BF16, tag="r0")
                        nc.scalar.activation(r0, vt, AF.Exp, scale=-0.5)
                        r1 = wk2.tile([128, 512], BF16, tag="r1")
                        nc.vector.tensor_scalar_mul(r1, r0, lw)
                        s1 = wk2.tile([128, 512], BF16, tag="s1")
                        nc.vector.scalar_tensor_tensor(s1, mups, -1.0, r1,
                                                       OP.mult, OP.mult)
                        t1 = wk2.tile([128, 512], BF16, tag="t1")
                        nc.vector.tensor_tensor(t1, sil[:, tsl], r1, OP.mult)
                        nc.vector.scalar_tensor_tensor(dst[:, tsl], t1, lb, s1,
                                                       OP.add, OP.add)

                kln_tm = wk1.tile([128, L], BF16, tag="klntm")
                for ci in range(NTC):
                    csl = slice(ci * 128, (ci + 1) * 128)
                    nc.scalar.dma_start_transpose(kln_tm[:, csl], kln[:, csl])

                gsil = wk1.tile([128, L], BF16, tag="gsil")
                for tt2 in range(4):
                    tsl = slice(tt2 * 512, (tt2 + 1) * 512)
                    pps = ps.tile([128, 512], F32, tag="ps")
                    for kk in range(NK):
                        nc.tensor.matmul(pps, wg_t[kk][:, hsl], hT_t[kk][:, tsl],
                                         start=(kk == 0), stop=(kk == NK - 1))
                    nc.scalar.activation(gsil[:, tsl], pps, AF.Silu)

                # ---- chunked scan ----
                S_bf = wk1.tile([128, 128], BF16, tag="sbf")
                nc.gpsimd.memset(S_bf, 0.0)
                og_h = wk1.tile([128, L], BF16, tag="ogh")
                for ci in range(NTC):
                    csl = slice(ci * 128, (ci + 1) * 128)
                    vsl = slice(ci * FSH + h * 128, ci * FSH + (h + 1) * 128)
                    spc = sp_all[:, ci * HL + h : ci * HL + h + 1]
                    spnc = spn_all[:, ci * HL + h : ci * HL + h + 1]

                    Yt = wk3.tile([128, 128], BF16, tag="Y")
                    nc.vector.tensor_scalar_mul(Yt, negu, spc)
                    gps = ps.tile([128, 128], F32, tag="ps")
                    nc.tensor.matmul(gps, ones_one, Yt, start=True, stop=False,
                                     skip_group_check=True)
                    gtps = ps.tile([128, 128], F32, tag="ps")
                    nc.tensor.matmul(gtps, Yt, ones_one, start=True, stop=True,
                                     skip_group_check=True)
                    ptile = wk3.tile([128, 128], F32, tag="pt")
                    nc.scalar.activation(ptile, gps, AF.Exp)
                    scol = wk3.tile([128, 1], F32, tag="scol")
                    nc.vector.scalar_tensor_tensor(scol, spnc, -1.0, gtps[:, 0:1],
                                                   OP.mult, OP.subtract)
                    nc.tensor.matmul(gps, ident, maskc, start=False, stop=True,
                                     skip_group_check=True)
                    dexp = wk3.tile([128, 128], F32, tag="dexp")
                    nc.scalar.activation(dexp, gps, AF.Exp, bias=scol)

                    atps = ps.tile([128, 128], F32, tag="ps")
                    nc.tensor.matmul(atps, kln[:, csl], qln[:, csl],
                                     start=True, stop=True)
                    ats = wk3.tile([128, 128], BF16, tag="ats")
                    nc.vector.tensor_tensor(ats, atps, dexp, OP.mult)
                    qs = wk3.tile([128, 128], BF16, tag="qs")
                    nc.vector.tensor_tensor(qs, qln[:, csl], ptile, OP.mult)
                    kw = wk3.tile([128, 128], BF16, tag="kw")
                    nc.vector.tensor_scalar_mul(kw, kln_tm[:, csl], dexp[:, 127:128])

                    ops_ = ps.tile([128, 128], F32, tag="ps")
                    nc.tensor.matmul(ops_, v_all[:, vsl], ats, start=True, stop=False,
                                     skip_group_check=True)
                    nc.tensor.matmul(ops_, S_bf, qs, start=False, stop=True,
                                     skip_group_check=True)
                    nc.vector.tensor_tensor(og_h[:, csl], ops_, gsil[:, csl], OP.mult)

                    sd = wk3.tile([128, 128], BF16, tag="sd")
                    nc.vector.tensor_scalar_mul(sd, ident, ptile[:, 127:128])
                    sps = ps.tile([128, 128], F32, tag="ps")
                    nc.tensor.matmul(sps, sd, S_bf, start=True, stop=False,
                                     skip_group_check=True)
                    nc.tensor.matmul(sps, kw, v_all[:, vsl], start=False, stop=True,
                                     skip_group_check=True)
                    nc.vector.tensor_copy(S_bf, sps)

                nc.sync.dma_start(out=og_d[h * 128 : (h + 1) * 128, :], in_=og_h)

        # ---- phase 3: AllGather ----------------------------------------
        nc.gpsimd.collective_compute(
            "AllGather", OP.bypass, replica_groups=GROUPS,
            ins=[og_d[:, :]], outs=[og_all[:, :]],
        )

    # ---- phase 4: out projection (wp freed) ----------------------------
    with tc.tile_pool(name="p4", bufs=1) as p4, \
         tc.tile_pool(name="p4b", bufs=3) as p4b:
        wo_t = []
        og_r = []
        for i in range(H * DV // 128):
            tt = p4.tile([128, FSH], BF16, tag=f"wo{i}")
            nc.sync.dma_start(out=tt, in_=wo[i * 128 : (i + 1) * 128, :])
            wo_t.append(tt)
            rr = p4.tile([128, L], BF16, tag=f"ogr{i}")
            nc.sync.dma_start(out=rr, in_=og_all[i * 128 : (i + 1) * 128, :])
            og_r.append(rr)
        for ci in range(NTC):
            csl = slice(ci * 128, (ci + 1) * 128)
            ops_ = ps.tile([128, FSH], F32, tag="ps")
            for ff in range(H * DV // 128):
                nc.tensor.matmul(ops_, og_r[ff][:, csl], wo_t[ff],
                                 start=(ff == 0), stop=(ff == H * DV // 128 - 1))
            outt = p4b.tile([128, FSH], F32, tag="outt")
            nc.scalar.activation(outt, ops_, AF.Copy)
            nc.sync.dma_start(out=out[csl, :], in_=outt)


def prep_inputs(inputs):
    """Shard/transpose/cast full inputs into 8 per-core in_maps."""
    f = {k: np.asarray(v, dtype=np.float32) for k, v in inputs.items()}
    h = f["hidden_states"]

    hT_b = [np.ascontiguousarray(h[b].T).astype(BF) for b in range(B)]
    in_maps = []
    for c in range(NCORES):
        b, r = c // 4, c % 4
        cols = slice(r * FSH, (r + 1) * FSH)

        def convw(w):
            m = w[cols].reshape(HL, 128, K).transpose(1, 0, 2).reshape(128, HL * K)
            return np.ascontiguousarray(m).astype(np.float32)

        def convb(bias):
            return np.ascontiguousarray(bias[cols].reshape(HL, 128).T).astype(np.float32)

        m = {
            "hT": hT_b[b],
            "wq": np.ascontiguousarray(f["Wq"][:, cols]).astype(BF),
            "wk": np.ascontiguousarray(f["Wk"][:, cols]).astype(BF),
            "wg": np.ascontiguousarray(f["Wg"][:, cols]).astype(BF),
            "wv": np.ascontiguousarray(f["Wv"][:, cols]).astype(BF),
            "wb": np.ascontiguousarray(f["Wb"][:, 4 * r : 4 * r + 4]).astype(BF),
            "wo": np.ascontiguousarray(f["Wo"][:, cols]).astype(BF),
            "qcw": convw(f["qconv_w"]),
            "kcw": convw(f["kconv_w"]),
            "qcb": convb(f["qconv_b"]),
            "kcb": convb(f["kconv_b"]),
            "qnw": f["qn_w"].reshape(128, 1).astype(np.float32),
            "qnb": f["qn_b"].reshape(128, 1).astype(np.float32),
            "knw": f["kn_w"].reshape(128, 1).astype(np.float32),
            "knb": f["kn_b"].reshape(128, 1).astype(np.float32),
            "bbb": np.ascontiguousarray(
                np.broadcast_to(f["bb"][4 * r : 4 * r + 4], (128, HL))
            ).astype(np.float32),
        }
        in_maps.append(m)
    return in_maps


_NC_CACHE = {}


def get_nc():
    if "nc" not in _NC_CACHE:
        _NC_CACHE["nc"] = build_kernel()
    return _NC_CACHE["nc"]


def assemble(results):
    full = np.empty((B, L, D), np.float32)
    for c in range(NCORES):
        b, r = c // 4, c % 4
        full[b][:, r * FSH : (r + 1) * FSH] = results[c]["out"]
    return full


def kernel(**inputs) -> np.ndarray:
    nc = get_nc()
    in_maps = prep_inputs(inputs)
    res = run_bass_kernel_spmd(nc, in_maps, list(range(NCORES)))
    return assemble(res.results)
